# revision 46
# baseline (speedup 1.0000x reference)
"""Trainium2 Bass kernel for batched multi-head attention (v2, all-bf16).

Full module:  out = softmax((X_q Wq)(X_k Wk)^T / sqrt(dh) + keymask) (X_v Wv) * qmask
Shapes: B=4, S=2048, D=1024, H=16, dh=64.

Sharding over 8 NeuronCores: core c -> (batch b = c//2, head-group g = c%2).
Each core computes batch b, heads g*8..g*8+8 (Wq/Wk/Wv column-sharded by head).
No collectives; the host scatters inputs and gathers the [2048, 512] output
blocks into the full [4, 2048, 1024] output.

Host-side marshaling: X tensors are transposed (X^T, contraction dim on
partitions) and cast to bf16; W column blocks cast to bf16; v_mask is folded
into X_v rows (numerator) and shipped as vmaskT (denominator column). This
removes all on-chip PE transposes of X and their PSUM evacuations.

Per-core schedule (all matmuls bf16, moving N=512):
  Phase 0: V projection + Q/K projections for head pair 0 (mc=0).
  Attention, one head PAIR at a time (heads 2i/2i+1 live on partition halves
  0:64 / 64:128 of the mc=i chunk of QW^T/KW^T):
    per kc: S^T for both heads -> one [128, 2, 512] PSUM tile via two
    CONCURRENT K=64 matmuls on PE array row-tiles (0,0)/(64,0);
    one ScalarE exp (N=1024, bf16 out) covers both heads;
    two K=128 AV matmuls accumulate O^T[65, 512] per head (row 64 = sum of
    exp * v_mask = softmax denominator).
  The exp stream is the bottleneck (~1.1us per kc); leftover PE time inside
  the loop is filled with the NEXT head pair's Q/K projection matmuls
  (pulled from a generator), so projections cost almost no wall time.
  Tails (PE-transpose O^T, normalize by qmask/denom, DMA out) are deferred
  into the next iteration's stream.
"""

import os
import sys
import time
import threading

for _p in ("/opt/trn_rl_repo", "/opt/pypackages"):
    if _p not in sys.path and os.path.isdir(_p):
        sys.path.append(_p)

import numpy as np
import ml_dtypes
from contextlib import ExitStack

import concourse.bass as bass
import concourse.tile as tile
from concourse import bacc, mybir
from concourse.bass_utils import run_bass_kernel_spmd
from concourse.masks import make_identity

B, S, D = 4, 2048, 1024
HEADS, DH = 16, 64
N_CORES = 8
HG = HEADS // 2          # 8 heads per core
MC = HG * DH             # 512 output cols per core
NSC = S // 128           # 16 seq chunks
NDC = D // 128           # 8 contraction chunks
NMC = MC // 128          # 4 head-dim chunks (= head pairs)
NKC = NSC                # 16 key chunks

F32 = mybir.dt.float32
BF16 = mybir.dt.bfloat16
EXP = mybir.ActivationFunctionType.Exp

QH = 512                 # q-half size
NQH = S // QH
QB = QH // 128
N_FILL = int(os.environ.get("N_FILL", "1"))   # filler units pulled per kc


def _emit(tc, t):
    nc = tc.nc
    ctx = ExitStack()

    # ---------------- persistent pools / DMAs ----------------
    cpool = ctx.enter_context(tc.tile_pool(name="const", bufs=1))
    x_pool = ctx.enter_context(tc.tile_pool(name="x", bufs=1))
    w_pool = ctx.enter_context(tc.tile_pool(name="w", bufs=1))

    # W first (small, needed by the first projections), then X^T quarters in
    # consumption order: xq/xk (phase-0 Q/K mc0) before xv (filler V proj).
    # Input DMAs split across the two HWDGE queues (sync + scalar) so the
    # q-side and k/v-side streams land in parallel; the scalar-queue DMAs
    # all precede the first exp.  Quarters are ordered by first consumption.
    w_sbs = {}
    for kind in ("q", "k"):
        w_sb = w_pool.tile([128, NDC, MC], BF16, name="w" + kind, tag="w" + kind)
        nc.sync.dma_start(w_sb[:], t["w" + kind].ap().rearrange("(dc p) m -> p dc m", p=128))
        w_sbs[kind] = w_sb
    xts = {}
    for name in ("xq", "xk", "xv"):
        xt = x_pool.tile([128, NDC, S], BF16, name=name + "t", tag=name + "t")
        xts[name] = xt
    x_views = {name: t[name].ap().rearrange("(dc p) s -> p dc s", p=128)
               for name in ("xq", "xk", "xv")}

    def x_dma(name, sq):
        nc.sync.dma_start(xts[name][:, :, sq * 512:(sq + 1) * 512],
                          x_views[name][:, :, sq * 512:(sq + 1) * 512])

    x_dma("xq", 0)
    x_dma("xk", 0)
    w_sb = w_pool.tile([128, NDC, MC], BF16, name="wv", tag="wv")
    nc.sync.dma_start(w_sb[:], t["wv"].ap().rearrange("(dc p) m -> p dc m", p=128))
    w_sbs["v"] = w_sb
    for name, sq in [("xv", 0), ("xv", 1), ("xk", 1), ("xv", 2), ("xk", 2),
                     ("xv", 3), ("xk", 3), ("xq", 1), ("xq", 2), ("xq", 3)]:
        x_dma(name, sq)

    ident = cpool.tile([128, 128], F32)
    make_identity(nc, ident[:])
    ident_b = cpool.tile([128, 128], BF16)
    nc.vector.tensor_copy(ident_b[:], ident[:])
    zbias = cpool.tile([128, 1], F32)
    nc.vector.memset(zbias[:], 0.0)
    qmaskT = cpool.tile([128, NSC], F32)
    nc.sync.dma_start(qmaskT[:], t["qmaskT"].ap())
    vmaskT = cpool.tile([128, NKC], BF16)
    nc.sync.dma_start(vmaskT[:], t["vmaskT"].ap())

    # exp table warmup while DMAs stream
    warm = cpool.tile([128, 1], BF16)
    nc.scalar.activation(warm[:], zbias[:], EXP, bias=zbias[:], scale=1.0)

    qk_pool = ctx.enter_context(tc.tile_pool(name="qk", bufs=1))
    qwT = qk_pool.tile([128, NMC, S], BF16)      # [m%128, mc, s]
    kwT = qk_pool.tile([128, NMC, S], BF16)
    vw = qk_pool.tile([128, NKC, HG, DH + 1], BF16)  # [k%128, kc, h, dh|m]
    nc.vector.tensor_copy(
        vw[:, :, :, DH:DH + 1],
        vmaskT[:].rearrange("p (k a b) -> p k a b", a=1, b=1)
        .broadcast_to([128, NKC, HG, 1]),
    )

    # ---------------- phase 0: Q/K mc=0 only ----------------
    pctx = ExitStack()
    psum_p = pctx.enter_context(tc.tile_pool(name="ps_p", bufs=2, space="PSUM"))

    def emit_v_group(sc, pool):
        """one [128,512] psum group of the V projection; generator-style"""
        pv = pool.tile([128, MC], F32, tag="f")
        for dc in range(NDC):
            nc.tensor.matmul(
                pv[:], xts["xv"][:, dc, sc * 128:(sc + 1) * 128],
                w_sbs["v"][:, dc, :],
                start=(dc == 0), stop=(dc == NDC - 1),
            )
            yield
        nc.vector.tensor_copy(
            vw[:, sc, :, 0:DH], pv[:].rearrange("p (h d) -> p h d", h=HG))
        yield

    def emit_qk_group(kind, mc, sq, pool, n=None):
        """one [128,512] psum group of the Q/K projection; generator-style"""
        dst = qwT if kind == "q" else kwT
        pp = pool.tile([128, 512], F32, tag="f")
        for dc in range(NDC):
            nc.tensor.matmul(
                pp[:], w_sbs[kind][:, dc, mc * 128:(mc + 1) * 128],
                xts["x" + kind][:, dc, sq * 512:(sq + 1) * 512],
                start=(dc == 0), stop=(dc == NDC - 1),
            )
            if n is not None:
                yield
        if kind == "q" and sq % 2 == 0:
            nc.scalar.copy(dst[:, mc, sq * 512:(sq + 1) * 512], pp[:])
        else:
            nc.vector.tensor_copy(dst[:, mc, sq * 512:(sq + 1) * 512], pp[:])
        if n is not None:
            yield

    # only the sq=0 groups up front -- the first q-half's S matmuls need
    # just qwT[mc0, 0:512] and kwT[mc0, 0:128]; the rest arrives as fillers
    for kind in ("q", "k"):
        for _ in emit_qk_group(kind, 0, 0, psum_p, n=None) or ():
            pass

    pctx.close()

    # ---------------- attention (+ projection fillers) ----------------
    actx = ExitStack()
    p_pool = actx.enter_context(tc.tile_pool(name="p", bufs=10))
    ot_pool = actx.enter_context(tc.tile_pool(name="ot", bufs=4))
    rq_pool = actx.enter_context(tc.tile_pool(name="rq", bufs=2))
    out_pool = actx.enter_context(tc.tile_pool(name="out", bufs=4))
    psum_s = actx.enter_context(tc.tile_pool(name="ps_s", bufs=2, space="PSUM"))
    psum_o = actx.enter_context(tc.tile_pool(name="ps_o", bufs=2, space="PSUM"))
    psum_f = actx.enter_context(tc.tile_pool(name="ps_f", bufs=2, space="PSUM"))

    # filler schedule: named groups in deadline order.  V{sc} feeds
    # AV(hp0,qh0,kc=sc); K{j}/Q{j} are the remaining mc0 s-quarters feeding
    # hp0's S matmuls (K: k-chunks 4j.., Q: q-half j); mc1-3 feed later
    # head pairs.
    GSZ = NDC + 1                       # matmuls + evac per group
    prelude = ["V0", "V1", "K1", "V2", "V3", "V4", "K2", "V5", "V6", "V7",
               "K3", "V8", "Q1", "V9", "V10", "Q2", "V11", "V12", "Q3",
               "V13", "V14", "V15"]
    order = prelude + [f"{kind}{mc}_{sq}" for mc in (1, 2, 3)
                       for kind in ("q", "k") for sq in range(4)]
    end_pos = {name: GSZ * (i + 1) for i, name in enumerate(order)}

    def filler_gen():
        for name in order:
            if name.startswith("V"):
                yield from emit_v_group(int(name[1:]), psum_f)
            elif name.startswith("K") or name.startswith("Q"):
                yield from emit_qk_group(name[0].lower(), 0, int(name[1:]),
                                         psum_f, n=1)
            else:
                yield from emit_qk_group(name[0], int(name[1]), int(name[3:]),
                                         psum_f, n=1)

    fill = filler_gen()
    pulled = [0]
    PRELUDE_UNITS = GSZ * len(prelude)
    UNITS_PER_MC = 2 * 4 * GSZ

    def pull(n):
        for _ in range(n):
            if next(fill, "done") == "done":
                break
            pulled[0] += 1

    def pull_to(target):
        pull(max(0, target - pulled[0]))

    out_v = t["out"].ap().rearrange(
        "(a qb p) (hh d) -> a p qb hh d", a=NQH, p=128, hh=HG)

    def emit_s_for(hp, qh, kc):
        # high priority: the exp stream's semaphore gates on the S pair's
        # position in the PE queue; sort it ahead of filler matmuls so exps
        # never wait behind DMA-gated projection work
        q0 = qh * QH
        s_ps = psum_s.tile([128, 2, QH], F32, tag="s")
        with tc.high_priority(offset=128):
            nc.tensor.matmul(
                s_ps[:, 0, :], kwT[0:64, hp, kc * 128:(kc + 1) * 128],
                qwT[0:64, hp, q0:q0 + QH], start=True, stop=True)
            nc.tensor.matmul(
                s_ps[:, 1, :], kwT[64:128, hp, kc * 128:(kc + 1) * 128],
                qwT[64:128, hp, q0:q0 + QH], start=True, stop=True)
        return s_ps

    def fill_target(hp):
        # fillers needed before head pair hp runs: prelude + mc chunks 1..hp
        return PRELUDE_UNITS + UNITS_PER_MC * hp if hp > 0 else 0

    pending_tail = []
    iters = [(hp, qh) for hp in range(NMC) for qh in range(NQH)]
    carry = []

    for it, (hp, qh) in enumerate(iters):
        o_lo = psum_o.tile([DH + 1, QH], F32, tag="o", name=f"olo{hp}_{qh}")
        o_hi = psum_o.tile([DH + 1, QH], F32, tag="o", name=f"ohi{hp}_{qh}")
        nxt = iters[it + 1] if it + 1 < len(iters) else None

        def emit_exp(s_ps):
            p_t = p_pool.tile([128, 2, QH], BF16, tag="p")
            with tc.high_priority(offset=16):
                nc.scalar.activation(
                    p_t[:], s_ps[:], EXP, bias=zbias[:], scale=0.125)
            return p_t

        def emit_av(kc, p_t, hp=hp, o_lo=o_lo, o_hi=o_hi):
            first, last = kc == 0, kc == NKC - 1
            with tc.high_priority(offset=32):
                nc.tensor.matmul(o_lo[:], vw[:, kc, 2 * hp, :], p_t[:, 0, :],
                                 start=first, stop=last)
                nc.tensor.matmul(o_hi[:], vw[:, kc, 2 * hp + 1, :], p_t[:, 1, :],
                                 start=first, stop=last)

        if carry:
            s_prev, s_cur = carry
            carry = []
        else:
            pull_to(fill_target(hp))
            s_prev = emit_s_for(hp, qh, 0)
            s_cur = emit_s_for(hp, qh, 1)
        if hp == 0 and qh == 0:
            pull_to(end_pos["V0"])

        # fillers are pulled at the END of each kc body: any matmul emitted
        # between an S pair and its exp inflates the PE-completion count the
        # exp's semaphore waits on, stalling the whole exp stream behind
        # filler work.
        for kc in range(NKC):
            p_t = emit_exp(s_prev)
            if 2 <= kc < 10 and pending_tail:
                pending_tail.pop(0)()
            emit_av(kc, p_t)
            s_prev = s_cur
            if kc + 2 < NKC:
                s_cur = emit_s_for(hp, qh, kc + 2)
            elif nxt is not None:
                # pre-emit the next iteration's first S pairs to keep the
                # exp stream gapless across (hp, qh) boundaries
                if nxt[0] != hp:
                    pull_to(fill_target(nxt[0]))
                elif nxt[0] == 0 and nxt[1] >= 1:
                    pull_to(end_pos[f"Q{nxt[1]}"])
                carry.append(emit_s_for(nxt[0], nxt[1], kc + 2 - NKC))
                s_cur = None
            if hp == 0 and qh == 0:
                # next AV needs V s-chunk kc+1; upcoming S pairs need their
                # kwT quarter
                tgt = end_pos[f"V{min(kc + 1, NKC - 1)}"]
                if kc + 3 < NKC and (kc + 3) // 4 > 0:
                    tgt = max(tgt, end_pos[f"K{(kc + 3) // 4}"])
                pull_to(tgt)
            elif nxt is not None and nxt[0] != hp:
                # last q-half before a head-pair switch: pull double so the
                # next mc chunk is stocked without a burst at the boundary
                pull(2)
            else:
                pull(N_FILL)

        while pending_tail:
            pending_tail.pop(0)()

        def make_tail(hp=hp, qh=qh, o_lo=o_lo, o_hi=o_hi):
            # evacuate PSUM accumulators immediately (frees the o slots for
            # the next iteration); defer the PE transposes + normalize.
            ots = {}
            for h, o_ps in ((2 * hp, o_lo), (2 * hp + 1, o_hi)):
                ot = ot_pool.tile([DH + 1, QH], BF16, tag="ot",
                                  name=f"ot_{h}_{qh}")
                nc.vector.tensor_copy(ot[:], o_ps[:])
                ots[h] = ot

            # each head's tail in three small steps so no single kc slot
            # absorbs a PE burst: transposes 0-1, transposes 2-3, normalize
            trs = {}

            def tr_step(h, half):
                if half == 0:
                    trs[h] = psum_f.tile([128, QB, DH + 2], BF16, tag="f",
                                         name=f"tr_{h}_{qh}")
                tr, ot = trs[h], ots[h]
                for qb in (2 * half, 2 * half + 1):
                    nc.tensor.transpose(
                        tr[:, qb, 0:DH + 1], ot[:, qb * 128:(qb + 1) * 128],
                        ident_b[0:DH + 1, 0:DH + 1])

            def norm_step(h):
                tr = trs[h]
                rq = rq_pool.tile([128, QB], F32, tag="rq",
                                  name=f"rq_{h}_{qh}")
                nc.vector.reciprocal(rq[:], tr[:, :, DH])
                nc.vector.tensor_mul(
                    rq[:], rq[:], qmaskT[:, qh * QB:(qh + 1) * QB])
                ob = out_pool.tile([128, QB, DH], F32, tag="ob",
                                   name=f"ob_{h}_{qh}")
                nc.vector.tensor_mul(
                    ob[:], tr[:, :, 0:DH], rq[:].broadcast_to([128, QB, DH]))
                nc.sync.dma_start(out_v[qh][:, :, h, :], ob[:])

            steps = []
            for h in (2 * hp, 2 * hp + 1):
                steps.append(lambda h=h: tr_step(h, 0))
                steps.append(lambda h=h: tr_step(h, 1))
                steps.append(lambda h=h: norm_step(h))
            return steps

        pending_tail.extend(make_tail())

    while pending_tail:
        pending_tail.pop(0)()
    pull(10 ** 9)
    actx.close()
    ctx.close()


_BUILD_LOCK = threading.Lock()
_CACHE = {}


def _build():
    with _BUILD_LOCK:
        if "nc" in _CACHE:
            return _CACHE["nc"]
        nc = bacc.Bacc(
            "TRN2", target_bir_lowering=False, debug=False, num_devices=N_CORES
        )
        t = {
            "xq": nc.dram_tensor("xq", [D, S], BF16, kind="ExternalInput"),
            "xk": nc.dram_tensor("xk", [D, S], BF16, kind="ExternalInput"),
            "xv": nc.dram_tensor("xv", [D, S], BF16, kind="ExternalInput"),
            "wq": nc.dram_tensor("wq", [D, MC], BF16, kind="ExternalInput"),
            "wk": nc.dram_tensor("wk", [D, MC], BF16, kind="ExternalInput"),
            "wv": nc.dram_tensor("wv", [D, MC], BF16, kind="ExternalInput"),
            "vmaskT": nc.dram_tensor("vmaskT", [128, NKC], BF16,
                                     kind="ExternalInput"),
            "qmaskT": nc.dram_tensor("qmaskT", [128, NSC], F32,
                                     kind="ExternalInput"),
            "out": nc.dram_tensor("out", [S, MC], F32, kind="ExternalOutput"),
        }
        with tile.TileContext(nc) as tc:
            _emit(tc, t)
        nc.compile()
        _CACHE["nc"] = nc
        return nc


def _in_maps(q_value, k_value, v_value, v_mask, q_mask, Wq, Wk, Wv):
    bf = ml_dtypes.bfloat16
    xqt, xkt, xvt = {}, {}, {}
    for b in range(B):
        xqt[b] = np.ascontiguousarray(q_value[b].T.astype(bf))
        xkt[b] = np.ascontiguousarray(k_value[b].T.astype(bf))
        # fold key mask into the V rows (numerator side)
        xvt[b] = np.ascontiguousarray((v_value[b] * v_mask[b]).T.astype(bf))
    w8 = {}
    for g in range(2):
        m0 = g * MC
        w8[g] = (np.ascontiguousarray(Wq[:, m0:m0 + MC].astype(bf)),
                 np.ascontiguousarray(Wk[:, m0:m0 + MC].astype(bf)),
                 np.ascontiguousarray(Wv[:, m0:m0 + MC].astype(bf)))
    maps = []
    for c in range(N_CORES):
        b, g = c // 2, c % 2
        vm = v_mask[b, :, 0].reshape(NKC, 128).T
        qm = q_mask[b, :, 0].reshape(NSC, 128).T
        maps.append({
            "xq": xqt[b], "xk": xkt[b], "xv": xvt[b],
            "wq": w8[g][0], "wk": w8[g][1], "wv": w8[g][2],
            "vmaskT": np.ascontiguousarray(vm.astype(bf)),
            "qmaskT": np.ascontiguousarray(qm).astype(np.float32),
        })
    return maps


def _assemble(results):
    out = np.empty((B, S, HEADS * DH), dtype=np.float32)
    for c in range(N_CORES):
        b, g = c // 2, c % 2
        out[b, :, g * MC:(g + 1) * MC] = results[c]["out"]
    return out


def kernel(q_value, k_value, v_value, v_mask, q_mask, Wq, Wk, Wv,
           profile=False, trace_cores=None):
    nc = _build()
    maps = _in_maps(np.asarray(q_value, dtype=np.float32),
                    np.asarray(k_value, dtype=np.float32),
                    np.asarray(v_value, dtype=np.float32),
                    np.asarray(v_mask, dtype=np.float32),
                    np.asarray(q_mask, dtype=np.float32),
                    np.asarray(Wq, dtype=np.float32),
                    np.asarray(Wk, dtype=np.float32),
                    np.asarray(Wv, dtype=np.float32))
    if profile:
        _install_profile_hook()
    res = run_bass_kernel_spmd(
        nc, maps, list(range(N_CORES)),
        trace=profile, trace_cores=trace_cores,
    )
    out = _assemble(res.results)
    if profile:
        return out, res
    return out


def _install_profile_hook():
    """Wire up the NTFF profile hook that this container image lacks."""
    import types
    if "antenv.axon_hooks" in sys.modules:
        return
    try:
        from trn_agent_boot.trn_boot import _ntff_profile_via_ctypes
        hook = _ntff_profile_via_ctypes("/opt/axon/libaxon_pjrt.so")
    except Exception:
        hook = None
    mod = types.ModuleType("antenv.axon_hooks")
    mod.get_axon_ntff_profile_hook = lambda: hook
    sys.modules["antenv.axon_hooks"] = mod


if __name__ == "__main__":
    t0 = time.time()
    _build()
    print(f"build+compile: {time.time() - t0:.1f}s")


# revision 47
# speedup vs baseline: 1.2010x; 1.2010x over previous
"""Trainium2 Bass kernel for batched multi-head attention (v2, all-bf16).

Full module:  out = softmax((X_q Wq)(X_k Wk)^T / sqrt(dh) + keymask) (X_v Wv) * qmask
Shapes: B=4, S=2048, D=1024, H=16, dh=64.

Sharding over 8 NeuronCores: core c -> (batch b = c//2, head-group g = c%2).
Each core computes batch b, heads g*8..g*8+8 (Wq/Wk/Wv column-sharded by head).
No collectives; the host scatters inputs and gathers the [2048, 512] output
blocks into the full [4, 2048, 1024] output.

Host-side marshaling: X tensors are transposed (X^T, contraction dim on
partitions) and cast to bf16; W column blocks cast to bf16; v_mask is folded
into X_v rows (numerator) and shipped as vmaskT (denominator column). This
removes all on-chip PE transposes of X and their PSUM evacuations.

Per-core schedule (all matmuls bf16, moving N=512):
  Phase 0: V projection + Q/K projections for head pair 0 (mc=0).
  Attention, one head PAIR at a time (heads 2i/2i+1 live on partition halves
  0:64 / 64:128 of the mc=i chunk of QW^T/KW^T):
    per kc: S^T for both heads -> one [128, 2, 512] PSUM tile via two
    CONCURRENT K=64 matmuls on PE array row-tiles (0,0)/(64,0);
    one ScalarE exp (N=1024, bf16 out) covers both heads;
    two K=128 AV matmuls accumulate O^T[65, 512] per head (row 64 = sum of
    exp * v_mask = softmax denominator).
  The exp stream is the bottleneck (~1.1us per kc); leftover PE time inside
  the loop is filled with the NEXT head pair's Q/K projection matmuls
  (pulled from a generator), so projections cost almost no wall time.
  Tails (PE-transpose O^T, normalize by qmask/denom, DMA out) are deferred
  into the next iteration's stream.
"""

import os
import sys
import time
import threading

for _p in ("/opt/trn_rl_repo", "/opt/pypackages"):
    if _p not in sys.path and os.path.isdir(_p):
        sys.path.append(_p)

import numpy as np
import ml_dtypes
from contextlib import ExitStack

import concourse.bass as bass
import concourse.tile as tile
from concourse import bacc, mybir
from concourse.bass_utils import run_bass_kernel_spmd
from concourse.masks import make_identity

B, S, D = 4, 2048, 1024
HEADS, DH = 16, 64
N_CORES = 8
HG = HEADS // 2          # 8 heads per core
MC = HG * DH             # 512 output cols per core
NSC = S // 128           # 16 seq chunks
NDC = D // 128           # 8 contraction chunks
NMC = MC // 128          # 4 head-dim chunks (= head pairs)
NKC = NSC                # 16 key chunks

F32 = mybir.dt.float32
BF16 = mybir.dt.bfloat16
EXP = mybir.ActivationFunctionType.Exp

QH = 512                 # q-half size
NQH = S // QH
QB = QH // 128
N_FILL = int(os.environ.get("N_FILL", "1"))   # filler units pulled per kc


def _emit(tc, t):
    nc = tc.nc
    ctx = ExitStack()

    # ---------------- persistent pools / DMAs ----------------
    cpool = ctx.enter_context(tc.tile_pool(name="const", bufs=1))
    x_pool = ctx.enter_context(tc.tile_pool(name="x", bufs=1))
    w_pool = ctx.enter_context(tc.tile_pool(name="w", bufs=1))

    # W first (small, needed by the first projections), then X^T quarters in
    # consumption order: xq/xk (phase-0 Q/K mc0) before xv (filler V proj).
    # Input DMAs split across the two HWDGE queues (sync + scalar) so the
    # q-side and k/v-side streams land in parallel; the scalar-queue DMAs
    # all precede the first exp.  Quarters are ordered by first consumption.
    w_sbs = {}
    for kind in ("q", "k"):
        w_sb = w_pool.tile([128, NDC, MC], BF16, name="w" + kind, tag="w" + kind)
        nc.sync.dma_start(w_sb[:], t["w" + kind].ap().rearrange("(dc p) m -> p dc m", p=128))
        w_sbs[kind] = w_sb
    xts = {}
    for name in ("xq", "xk", "xv"):
        xt = x_pool.tile([128, NDC, S], BF16, name=name + "t", tag=name + "t")
        xts[name] = xt
    x_views = {name: t[name].ap().rearrange("(dc p) s -> p dc s", p=128)
               for name in ("xq", "xk", "xv")}

    def x_dma(name, sq):
        nc.sync.dma_start(xts[name][:, :, sq * 512:(sq + 1) * 512],
                          x_views[name][:, :, sq * 512:(sq + 1) * 512])

    x_dma("xq", 0)
    x_dma("xk", 0)
    w_sb = w_pool.tile([128, NDC, MC], BF16, name="wv", tag="wv")
    nc.sync.dma_start(w_sb[:], t["wv"].ap().rearrange("(dc p) m -> p dc m", p=128))
    w_sbs["v"] = w_sb
    for name, sq in [("xv", 0), ("xv", 1), ("xk", 1), ("xv", 2), ("xk", 2),
                     ("xv", 3), ("xk", 3), ("xq", 1), ("xq", 2), ("xq", 3)]:
        x_dma(name, sq)

    ident = cpool.tile([128, 128], F32)
    make_identity(nc, ident[:])
    ident_b = cpool.tile([128, 128], BF16)
    nc.vector.tensor_copy(ident_b[:], ident[:])
    zbias = cpool.tile([128, 1], F32)
    nc.vector.memset(zbias[:], 0.0)
    qmaskT = cpool.tile([128, NSC], F32)
    nc.sync.dma_start(qmaskT[:], t["qmaskT"].ap())
    vmaskT = cpool.tile([128, NKC], BF16)
    nc.sync.dma_start(vmaskT[:], t["vmaskT"].ap())

    # exp table warmup while DMAs stream
    warm = cpool.tile([128, 1], BF16)
    nc.scalar.activation(warm[:], zbias[:], EXP, bias=zbias[:], scale=1.0)

    qk_pool = ctx.enter_context(tc.tile_pool(name="qk", bufs=1))
    qwT = qk_pool.tile([128, NMC, S], BF16)      # [m%128, mc, s]
    kwT = qk_pool.tile([128, NMC, S], BF16)
    vw = qk_pool.tile([128, NKC, HG, DH + 1], BF16)  # [k%128, kc, h, dh|m]
    nc.vector.tensor_copy(
        vw[:, :, :, DH:DH + 1],
        vmaskT[:].rearrange("p (k a b) -> p k a b", a=1, b=1)
        .broadcast_to([128, NKC, HG, 1]),
    )

    # ---------------- phase 0: Q/K mc=0 only ----------------
    pctx = ExitStack()
    psum_p = pctx.enter_context(tc.tile_pool(name="ps_p", bufs=2, space="PSUM"))

    def emit_v_group(sc, pool):
        """one [128,512] psum group of the V projection; generator-style"""
        pv = pool.tile([128, MC], F32, tag="f")
        for dc in range(NDC):
            nc.tensor.matmul(
                pv[:], xts["xv"][:, dc, sc * 128:(sc + 1) * 128],
                w_sbs["v"][:, dc, :],
                start=(dc == 0), stop=(dc == NDC - 1),
            )
            yield
        nc.vector.tensor_copy(
            vw[:, sc, :, 0:DH], pv[:].rearrange("p (h d) -> p h d", h=HG))
        yield

    def emit_qk_group(kind, mc, sq, pool, n=None):
        """one [128,512] psum group of the Q/K projection; generator-style"""
        dst = qwT if kind == "q" else kwT
        pp = pool.tile([128, 512], F32, tag="f")
        for dc in range(NDC):
            nc.tensor.matmul(
                pp[:], w_sbs[kind][:, dc, mc * 128:(mc + 1) * 128],
                xts["x" + kind][:, dc, sq * 512:(sq + 1) * 512],
                start=(dc == 0), stop=(dc == NDC - 1),
            )
            if n is not None:
                yield
        if kind == "q" and sq % 2 == 0:
            nc.scalar.copy(dst[:, mc, sq * 512:(sq + 1) * 512], pp[:])
        else:
            nc.vector.tensor_copy(dst[:, mc, sq * 512:(sq + 1) * 512], pp[:])
        if n is not None:
            yield

    # only the sq=0 groups up front -- the first q-half's S matmuls need
    # just qwT[mc0, 0:512] and kwT[mc0, 0:128]; the rest arrives as fillers
    for kind in ("q", "k"):
        for _ in emit_qk_group(kind, 0, 0, psum_p, n=None) or ():
            pass

    pctx.close()

    # ---------------- attention (+ projection fillers) ----------------
    actx = ExitStack()
    p_pool = actx.enter_context(tc.tile_pool(name="p", bufs=10))
    ot_pool = actx.enter_context(tc.tile_pool(name="ot", bufs=4))
    rq_pool = actx.enter_context(tc.tile_pool(name="rq", bufs=2))
    out_pool = actx.enter_context(tc.tile_pool(name="out", bufs=4))
    psum_s = actx.enter_context(tc.tile_pool(name="ps_s", bufs=2, space="PSUM"))
    psum_o = actx.enter_context(tc.tile_pool(name="ps_o", bufs=2, space="PSUM"))
    psum_f = actx.enter_context(tc.tile_pool(name="ps_f", bufs=2, space="PSUM"))

    # filler schedule: named groups in deadline order.  V{sc} feeds
    # AV(hp0,qh0,kc=sc); K{j}/Q{j} are the remaining mc0 s-quarters feeding
    # hp0's S matmuls (K: k-chunks 4j.., Q: q-half j); mc1-3 feed later
    # head pairs.
    GSZ = NDC + 1                       # matmuls + evac per group
    prelude = ["V0", "V1", "K1", "V2", "V3", "V4", "K2", "V5", "V6", "V7",
               "K3", "V8", "Q1", "V9", "V10", "Q2", "V11", "V12", "Q3",
               "V13", "V14", "V15"]
    order = prelude + [f"{kind}{mc}_{sq}" for mc in (1, 2, 3)
                       for kind in ("q", "k") for sq in range(4)]
    end_pos = {name: GSZ * (i + 1) for i, name in enumerate(order)}

    def filler_gen():
        for name in order:
            if name.startswith("V"):
                yield from emit_v_group(int(name[1:]), psum_f)
            elif name.startswith("K") or name.startswith("Q"):
                yield from emit_qk_group(name[0].lower(), 0, int(name[1:]),
                                         psum_f, n=1)
            else:
                yield from emit_qk_group(name[0], int(name[1]), int(name[3:]),
                                         psum_f, n=1)

    fill = filler_gen()
    pulled = [0]
    PRELUDE_UNITS = GSZ * len(prelude)
    UNITS_PER_MC = 2 * 4 * GSZ

    def pull(n):
        for _ in range(n):
            if next(fill, "done") == "done":
                break
            pulled[0] += 1

    def pull_to(target):
        pull(max(0, target - pulled[0]))

    out_v = t["out"].ap().rearrange(
        "(a qb p) (hh d) -> a p qb hh d", a=NQH, p=128, hh=HG)

    def emit_s_for(hp, qh, kc):
        # high priority: the exp stream's semaphore gates on the S pair's
        # position in the PE queue; sort it ahead of filler matmuls so exps
        # never wait behind DMA-gated projection work
        q0 = qh * QH
        s_ps = psum_s.tile([128, 2, QH], F32, tag="s")
        with tc.high_priority(offset=128):
            nc.tensor.matmul(
                s_ps[:, 0, :], kwT[0:64, hp, kc * 128:(kc + 1) * 128],
                qwT[0:64, hp, q0:q0 + QH], start=True, stop=True)
            nc.tensor.matmul(
                s_ps[:, 1, :], kwT[64:128, hp, kc * 128:(kc + 1) * 128],
                qwT[64:128, hp, q0:q0 + QH], start=True, stop=True)
        return s_ps

    def fill_target(hp):
        # fillers needed before head pair hp runs: prelude + mc chunks 1..hp
        return PRELUDE_UNITS + UNITS_PER_MC * hp if hp > 0 else 0

    pending_tail = []
    iters = [(hp, qh) for hp in range(NMC) for qh in range(NQH)]
    carry = []

    for it, (hp, qh) in enumerate(iters):
        o_lo = psum_o.tile([DH + 1, QH], F32, tag="o", name=f"olo{hp}_{qh}")
        o_hi = psum_o.tile([DH + 1, QH], F32, tag="o", name=f"ohi{hp}_{qh}")
        nxt = iters[it + 1] if it + 1 < len(iters) else None

        def emit_exp(s_ps):
            p_t = p_pool.tile([128, 2, QH], BF16, tag="p")
            nc.scalar.activation(
                p_t[:], s_ps[:], EXP, bias=zbias[:], scale=0.125)
            return p_t

        def emit_av(kc, p_t, hp=hp, o_lo=o_lo, o_hi=o_hi):
            first, last = kc == 0, kc == NKC - 1
            nc.tensor.matmul(o_lo[:], vw[:, kc, 2 * hp, :], p_t[:, 0, :],
                             start=first, stop=last)
            nc.tensor.matmul(o_hi[:], vw[:, kc, 2 * hp + 1, :], p_t[:, 1, :],
                             start=first, stop=last)

        if carry:
            s_prev, s_cur = carry
            carry = []
        else:
            pull_to(fill_target(hp))
            s_prev = emit_s_for(hp, qh, 0)
            s_cur = emit_s_for(hp, qh, 1)
        if hp == 0 and qh == 0:
            pull_to(end_pos["V0"])

        # fillers are pulled at the END of each kc body: any matmul emitted
        # between an S pair and its exp inflates the PE-completion count the
        # exp's semaphore waits on, stalling the whole exp stream behind
        # filler work.
        for kc in range(NKC):
            p_t = emit_exp(s_prev)
            if 2 <= kc < 10 and pending_tail:
                pending_tail.pop(0)()
            emit_av(kc, p_t)
            s_prev = s_cur
            if kc + 2 < NKC:
                s_cur = emit_s_for(hp, qh, kc + 2)
            elif nxt is not None:
                # pre-emit the next iteration's first S pairs to keep the
                # exp stream gapless across (hp, qh) boundaries
                if nxt[0] != hp:
                    pull_to(fill_target(nxt[0]))
                elif nxt[0] == 0 and nxt[1] >= 1:
                    pull_to(end_pos[f"Q{nxt[1]}"])
                carry.append(emit_s_for(nxt[0], nxt[1], kc + 2 - NKC))
                s_cur = None
            if hp == 0 and qh == 0:
                # next AV needs V s-chunk kc+1; upcoming S pairs need their
                # kwT quarter
                tgt = end_pos[f"V{min(kc + 1, NKC - 1)}"]
                if kc + 3 < NKC and (kc + 3) // 4 > 0:
                    tgt = max(tgt, end_pos[f"K{(kc + 3) // 4}"])
                pull_to(tgt)
            elif nxt is not None and nxt[0] != hp:
                # last q-half before a head-pair switch: pull double so the
                # next mc chunk is stocked without a burst at the boundary
                pull(2)
            else:
                pull(N_FILL)

        while pending_tail:
            pending_tail.pop(0)()

        def make_tail(hp=hp, qh=qh, o_lo=o_lo, o_hi=o_hi):
            # evacuate PSUM accumulators immediately (frees the o slots for
            # the next iteration); defer the PE transposes + normalize.
            ots = {}
            for h, o_ps in ((2 * hp, o_lo), (2 * hp + 1, o_hi)):
                ot = ot_pool.tile([DH + 1, QH], BF16, tag="ot",
                                  name=f"ot_{h}_{qh}")
                nc.vector.tensor_copy(ot[:], o_ps[:])
                ots[h] = ot

            # each head's tail in three small steps so no single kc slot
            # absorbs a PE burst: transposes 0-1, transposes 2-3, normalize
            trs = {}

            def tr_step(h, half):
                if half == 0:
                    trs[h] = psum_f.tile([128, QB, DH + 2], BF16, tag="f",
                                         name=f"tr_{h}_{qh}")
                tr, ot = trs[h], ots[h]
                for qb in (2 * half, 2 * half + 1):
                    nc.tensor.transpose(
                        tr[:, qb, 0:DH + 1], ot[:, qb * 128:(qb + 1) * 128],
                        ident_b[0:DH + 1, 0:DH + 1])

            def norm_step(h):
                tr = trs[h]
                rq = rq_pool.tile([128, QB], F32, tag="rq",
                                  name=f"rq_{h}_{qh}")
                nc.vector.reciprocal(rq[:], tr[:, :, DH])
                nc.vector.tensor_mul(
                    rq[:], rq[:], qmaskT[:, qh * QB:(qh + 1) * QB])
                ob = out_pool.tile([128, QB, DH], F32, tag="ob",
                                   name=f"ob_{h}_{qh}")
                nc.vector.tensor_mul(
                    ob[:], tr[:, :, 0:DH], rq[:].broadcast_to([128, QB, DH]))
                nc.sync.dma_start(out_v[qh][:, :, h, :], ob[:])

            steps = []
            for h in (2 * hp, 2 * hp + 1):
                steps.append(lambda h=h: tr_step(h, 0))
                steps.append(lambda h=h: tr_step(h, 1))
                steps.append(lambda h=h: norm_step(h))
            return steps

        pending_tail.extend(make_tail())

    while pending_tail:
        pending_tail.pop(0)()
    pull(10 ** 9)
    actx.close()
    ctx.close()


_BUILD_LOCK = threading.Lock()
_CACHE = {}


def _build():
    with _BUILD_LOCK:
        if "nc" in _CACHE:
            return _CACHE["nc"]
        nc = bacc.Bacc(
            "TRN2", target_bir_lowering=False, debug=False, num_devices=N_CORES
        )
        t = {
            "xq": nc.dram_tensor("xq", [D, S], BF16, kind="ExternalInput"),
            "xk": nc.dram_tensor("xk", [D, S], BF16, kind="ExternalInput"),
            "xv": nc.dram_tensor("xv", [D, S], BF16, kind="ExternalInput"),
            "wq": nc.dram_tensor("wq", [D, MC], BF16, kind="ExternalInput"),
            "wk": nc.dram_tensor("wk", [D, MC], BF16, kind="ExternalInput"),
            "wv": nc.dram_tensor("wv", [D, MC], BF16, kind="ExternalInput"),
            "vmaskT": nc.dram_tensor("vmaskT", [128, NKC], BF16,
                                     kind="ExternalInput"),
            "qmaskT": nc.dram_tensor("qmaskT", [128, NSC], F32,
                                     kind="ExternalInput"),
            "out": nc.dram_tensor("out", [S, MC], F32, kind="ExternalOutput"),
        }
        with tile.TileContext(nc) as tc:
            _emit(tc, t)
        nc.compile()
        _CACHE["nc"] = nc
        return nc


def _in_maps(q_value, k_value, v_value, v_mask, q_mask, Wq, Wk, Wv):
    bf = ml_dtypes.bfloat16
    xqt, xkt, xvt = {}, {}, {}
    for b in range(B):
        xqt[b] = np.ascontiguousarray(q_value[b].T.astype(bf))
        xkt[b] = np.ascontiguousarray(k_value[b].T.astype(bf))
        # fold key mask into the V rows (numerator side)
        xvt[b] = np.ascontiguousarray((v_value[b] * v_mask[b]).T.astype(bf))
    w8 = {}
    for g in range(2):
        m0 = g * MC
        w8[g] = (np.ascontiguousarray(Wq[:, m0:m0 + MC].astype(bf)),
                 np.ascontiguousarray(Wk[:, m0:m0 + MC].astype(bf)),
                 np.ascontiguousarray(Wv[:, m0:m0 + MC].astype(bf)))
    maps = []
    for c in range(N_CORES):
        b, g = c // 2, c % 2
        vm = v_mask[b, :, 0].reshape(NKC, 128).T
        qm = q_mask[b, :, 0].reshape(NSC, 128).T
        maps.append({
            "xq": xqt[b], "xk": xkt[b], "xv": xvt[b],
            "wq": w8[g][0], "wk": w8[g][1], "wv": w8[g][2],
            "vmaskT": np.ascontiguousarray(vm.astype(bf)),
            "qmaskT": np.ascontiguousarray(qm).astype(np.float32),
        })
    return maps


def _assemble(results):
    out = np.empty((B, S, HEADS * DH), dtype=np.float32)
    for c in range(N_CORES):
        b, g = c // 2, c % 2
        out[b, :, g * MC:(g + 1) * MC] = results[c]["out"]
    return out


def kernel(q_value, k_value, v_value, v_mask, q_mask, Wq, Wk, Wv,
           profile=False, trace_cores=None):
    nc = _build()
    maps = _in_maps(np.asarray(q_value, dtype=np.float32),
                    np.asarray(k_value, dtype=np.float32),
                    np.asarray(v_value, dtype=np.float32),
                    np.asarray(v_mask, dtype=np.float32),
                    np.asarray(q_mask, dtype=np.float32),
                    np.asarray(Wq, dtype=np.float32),
                    np.asarray(Wk, dtype=np.float32),
                    np.asarray(Wv, dtype=np.float32))
    if profile:
        _install_profile_hook()
    res = run_bass_kernel_spmd(
        nc, maps, list(range(N_CORES)),
        trace=profile, trace_cores=trace_cores,
    )
    out = _assemble(res.results)
    if profile:
        return out, res
    return out


def _install_profile_hook():
    """Wire up the NTFF profile hook that this container image lacks."""
    import types
    if "antenv.axon_hooks" in sys.modules:
        return
    try:
        from trn_agent_boot.trn_boot import _ntff_profile_via_ctypes
        hook = _ntff_profile_via_ctypes("/opt/axon/libaxon_pjrt.so")
    except Exception:
        hook = None
    mod = types.ModuleType("antenv.axon_hooks")
    mod.get_axon_ntff_profile_hook = lambda: hook
    sys.modules["antenv.axon_hooks"] = mod


if __name__ == "__main__":
    t0 = time.time()
    _build()
    print(f"build+compile: {time.time() - t0:.1f}s")
